# revision 32
# baseline (speedup 1.0000x reference)
"""Multi-head self-attention (b=4, L=2048, d=512, h=8) on 8 trn2 cores.

Sharding: data-parallel over batch (4) x tensor-parallel over heads (2 groups
of 4).  Core c handles batch c//2, heads [4*(c%2), 4*(c%2)+4).  Each core
returns a partial output (row-parallel Wo); the host sums the two partials per
batch and adds bo.

Device-side layout (all "transposed", so no on-device transposes are needed):
  xT   [512 d, 2048 q]   (host passes x[b].T, bf16)
  Q^T  [256 hd, 2048 q]  = WqT.T @ xT   (lhsT = WqT[d, hd] natural)
  K^T  same
  V    [2048 k, 256 hd]  (lhsT = xT chunks)  + ones column per head -> V_aug
  S^T  [128 k-tile, 512 q] = K^T_h.T-contraction(d_head=64)  -> PE 64-row mode,
       two heads of a pair run on independent half-arrays (T0 / T8)
  E^T  = exp(S^T * scale)  on ACT, one 1024-wide call per (k-tile, head-pair)
  O^T_aug [65, 512q] += V_aug_h.T @ E^T_h, giving O = P@V un-normalized plus
       the softmax denominator d[q] in row 64 (from the ones column).
  normalize: O = O[0:64] * broadcast(1/d)  (DVE; bv is folded into the host
       gather since softmax rows sum to 1: partial includes no bias, host
       adds bo + bv @ Wo.T)
  outT [512 e, 2048 q] = WoT.T @ O_norm^T  (fp16 partial, host sums)

Scheduling (the steady state runs the ACT engine ~98% busy at ~1.12us per
k-tile with the PE saturated underneath):
  - input DMAs split across the sync/scalar/gpsimd HW queues, ordered so the
    critical path (wk, wq, x chunk 0) lands first; x0 is split by
    contraction halves so the first K/Q projection halves start early
  - ~5us of tiny PE warmup matmuls bridge the DMA wait so the HAM clock
    gate is at 2.4 GHz when the first projection issues
  - all filler work (V/K/Q projections, previous chunk's output projection)
    is emitted AFTER each tile's scores/exp/PV so the FIFO engine queues
    serve the critical loop first; projection units are emitted in halves
  - each tile's PV pair is emitted one tile late (after the next tile's
    scores+exp) so ACT never waits on PE PV drain at block boundaries
  - PSUM o accumulators are evacuated to SBUF immediately at block end
    (both heads first), the 1/d normalize chain then runs lazily off the
    critical path
  - the final block's normalize + output projection run in q-halves with
    casts split between DVE and the idle ACT engine and out-DMAs spread
    over two queues to compress the tail
"""

import numpy as np
import ml_dtypes

import concourse.bass as bass
import concourse.bacc as bacc
import concourse.tile as tile
import concourse.mybir as mybir
from concourse.bass_utils import run_bass_kernel_spmd

F32 = mybir.dt.float32
F16 = mybir.dt.float16
BF16 = mybir.dt.bfloat16

B, L, D = 4, 2048, 512
NH, DH = 8, 64
HG = 2                 # head groups (tensor parallel)
GH = NH // HG          # 4 heads per group
EG = GH * DH           # 256 columns per group
SCALE = 1.0 / float(np.sqrt(DH))
P = 128
KT = L // P            # 16 k-tiles
QC = L // 512          # 4 q-chunks of 512
DC = D // P            # 4 d-chunks

_ts = bass.ts


def _body(tc):
    nc = tc.nc
    xT = nc.dram_tensor("xT", [D, L], BF16, kind="ExternalInput")
    wqT = nc.dram_tensor("wqT", [D, EG], BF16, kind="ExternalInput")
    wkT = nc.dram_tensor("wkT", [D, EG], BF16, kind="ExternalInput")
    wvT = nc.dram_tensor("wvT", [D, EG], BF16, kind="ExternalInput")
    woT = nc.dram_tensor("woT", [EG, D], BF16, kind="ExternalInput")
    # packed biases: col 0,1 = bq (t0,t1); col 2,3 = bk.  (bv is folded into
    # the host-side gather: (O/d + bv) @ Wo = (O/d) @ Wo + bv @ Wo, and
    # bv @ Wo.T is a constant [512] vector added with bo.)
    bpk = nc.dram_tensor("bpk", [P, 4], F32, kind="ExternalInput")
    outT = nc.dram_tensor("outT", [D, L], F16, kind="ExternalOutput")

    add = mybir.AluOpType.add
    Exp = mybir.ActivationFunctionType.Exp

    with (
        tc.tile_pool(name="const", bufs=1) as const,
        # one PSUM pool for the whole kernel so the phases can overlap:
        #   tag "s": score tiles [128,2,512] (2 banks) x2  -> 4 banks
        #   tag "o": PV accumulators [65,512] x2           -> 2 banks
        #   tag "u": proj / out-proj psum [128,512] x2     -> 2 banks
        tc.tile_pool(name="ps", bufs=2, space="PSUM") as ps,
        tc.tile_pool(name="ew", bufs=3) as ew,
        tc.tile_pool(name="nw", bufs=3) as nw,
        tc.tile_pool(name="ow", bufs=4) as ow,
    ):
        # ---- load inputs; order == Sync-queue issue order == transfer order.
        # Critical path to the first exp: wk(t0), x(n0), wq(t0), biases.
        wq_sb = const.tile([P, DC, EG], BF16)
        wk_sb = const.tile([P, DC, EG], BF16)
        wv_sb = const.tile([P, DC, EG], BF16)
        xT_sb = const.tile([P, DC, L], BF16)
        wo_sb = const.tile([P, HG, D], BF16)
        bias_sb = const.tile([P, 4], F32)

        def w_load(eng, w_sb, w_dr, t):
            eng.dma_start(
                out=w_sb[:, :, _ts(t, P)],
                in_=w_dr[:, _ts(t, P)].rearrange("(c p) e -> p c e", p=P))

        def x_load(eng, n, cs=slice(0, DC)):
            eng.dma_start(
                out=xT_sb[:, cs, _ts(n, 512)],
                in_=xT[:, _ts(n, 512)].rearrange("(c p) k -> p c k", p=P)[:, cs, :])

        # input loads spread across three idle engine DMA queues so the
        # descriptors issue (and transfers start) in parallel; critical path
        # to the first exp is wk, wq, x0(c=0,1)
        wv_r = wvT[:, :].rearrange("(c p) e -> p c e", p=P)
        w_load(nc.sync, wk_sb, wkT, 0)
        w_load(nc.scalar, wq_sb, wqT, 0)
        nc.gpsimd.dma_start(out=bias_sb[:], in_=bpk[:, :])
        # x0 split across both queues so its halves transfer in parallel
        x_load(nc.sync, 0, slice(0, 1))
        x_load(nc.scalar, 0, slice(1, 2))
        x_load(nc.sync, 0, slice(2, 3))
        x_load(nc.scalar, 0, slice(3, 4))
        nc.sync.dma_start(out=wv_sb[:, 0:2, :], in_=wv_r[:, 0:2, :])
        nc.scalar.dma_start(out=wv_sb[:, 2:4, :], in_=wv_r[:, 2:4, :])
        x_load(nc.sync, 1)
        x_load(nc.scalar, 2)
        w_load(nc.sync, wk_sb, wkT, 1)
        w_load(nc.scalar, wq_sb, wqT, 1)
        x_load(nc.sync, 3)
        nc.scalar.dma_start(
            out=wo_sb[:], in_=woT[:, :].rearrange("(c p) e -> p c e", p=P))

        bq_sb = bias_sb[:, 0:2]
        bk_sb = bias_sb[:, 2:4]

        # persistent activations
        qt_sb = const.tile([P, HG, L], BF16)     # Q^T  rows: head 2t + (r//64)
        kt_sb = const.tile([P, HG, L], BF16)     # K^T
        va_sb = const.tile([P, KT, GH, DH + 1], BF16)  # V + ones col, per k-tile
        on_sb = const.tile([P, HG, L], BF16)     # normalized O^T (attn output)

        nc.vector.memset(va_sb[:, :, :, DH:DH + 1], 1.0)

        # PE warmup: ~4.3us of full-array matmuls while the input DMAs are
        # in flight, so the HAM clock gate reaches 2.4 GHz before the first
        # projection (cold PE runs at 1.2 GHz; the HAM watches array
        # activity, so the warmup must light up the whole 128x128 array)
        warm = const.tile([P, 512], BF16)
        nc.vector.memset(warm[:], 0.0)
        wps = ps.tile([P, 512], F32, tag="u", name="warm_ps")
        for _ in range(60):
            nc.tensor.matmul(wps[:, 0:96], warm[:, 0:P], warm[:, 0:96],
                             start=True, stop=True)

        # ---- projections (128-row PE mode), emitted just-in-time ----------
        # Split into halves so an injected unit never monopolizes the PE
        # queue ahead of the next score tile.
        proj_ps = {}

        def proj_half(w_sb, dst, b_sb, t, n, half, on_act=False):
            key = (id(w_sb), t, n)
            if half == 0:
                proj_ps[key] = ps.tile([P, 512], F32, tag="u",
                                       name=f"p_ps_{t}_{n}")
            psq = proj_ps[key]
            for c in (0, 1) if half == 0 else (2, 3):
                nc.tensor.matmul(
                    psq[:],
                    w_sb[:, c, _ts(t, P)],
                    xT_sb[:, c, _ts(n, 512)],
                    start=(c == 0), stop=(c == DC - 1),
                )
            if half == 1:
                if on_act:
                    nc.scalar.add(dst[:, t, _ts(n, 512)], psq[:],
                                  b_sb[:, t:t + 1])
                else:
                    nc.vector.tensor_scalar(
                        out=dst[:, t, _ts(n, 512)], in0=psq[:],
                        scalar1=b_sb[:, t:t + 1], scalar2=None, op0=add,
                    )

        def proj_unit(w_sb, dst, b_sb, t, n):
            proj_half(w_sb, dst, b_sb, t, n, 0)
            proj_half(w_sb, dst, b_sb, t, n, 1)

        def v_proj(i):
            psv = ps.tile([P, EG], F32, tag="u", name=f"v_ps_{i}")
            for c in range(DC):
                nc.tensor.matmul(
                    psv[:],
                    xT_sb[:, c, _ts(i, P)],
                    wv_sb[:, c, :],
                    start=(c == 0), stop=(c == DC - 1),
                )
            nc.vector.tensor_copy(
                out=va_sb[:, i, :, 0:DH],
                in_=psv[:].rearrange("p (h d) -> p h d", d=DH),
            )

        def v_proj_half(i, half):
            key = ("v", i)
            if half == 0:
                proj_ps[key] = ps.tile([P, EG], F32, tag="u",
                                       name=f"v_ps_{i}")
            psv = proj_ps[key]
            for c in (0, 1) if half == 0 else (2, 3):
                nc.tensor.matmul(
                    psv[:],
                    xT_sb[:, c, _ts(i, P)],
                    wv_sb[:, c, :],
                    start=(c == 0), stop=(c == DC - 1),
                )
            if half == 1:
                nc.vector.tensor_copy(
                    out=va_sb[:, i, :, 0:DH],
                    in_=psv[:].rearrange("p (h d) -> p h d", d=DH),
                )

        def out_proj_m(n, m, nq=512, qh=0, eng=None, cast_on_act=False):
            eng = eng or nc.sync
            # one e-tile of the output projection for q-chunk n
            pso = ps.tile([P, 512], F32, tag="u", name=f"o_ps_{m}_{n}_{qh}",
                          padded_shape=[P, 512])
            qs = slice(n * 512 + qh * nq, n * 512 + qh * nq + nq)
            for c in range(HG):
                nc.tensor.matmul(
                    pso[:, 0:nq],
                    wo_sb[:, c, _ts(m, P)],
                    on_sb[:, c, qs],
                    start=(c == 0), stop=(c == HG - 1),
                )
            st = ow.tile([P, 512], F16, tag="o_st", padded_shape=[P, 512],
                         name=f"o_st_{m}_{n}_{qh}")
            if cast_on_act:
                nc.scalar.copy(st[:, 0:nq], pso[:, 0:nq])
            else:
                nc.vector.tensor_copy(st[:, 0:nq], pso[:, 0:nq])
            eng.dma_start(
                out=outT[:, :][_ts(m, P), qs], in_=st[:, 0:nq],
            )

        # Upfront: only what the first k-tiles of block (0,0) need.  Emitted
        # as interleaved halves so the K half on x(c=0,1) starts while
        # x(c=2,3) is still in flight.
        proj_half(wk_sb, kt_sb, bk_sb, 0, 0, 0)
        proj_half(wq_sb, qt_sb, bq_sb, 0, 0, 0)
        proj_half(wk_sb, kt_sb, bk_sb, 0, 0, 1, on_act=True)
        proj_half(wq_sb, qt_sb, bq_sb, 0, 0, 1)

        def inject(n, pr, i):
            """Emit filler work (projections, previous chunk's output
            projection) AFTER tile i's attention ops so the FIFO PE queue
            serves scores/PV first and the filler soaks up ACT-bound slack."""
            if n == 0 and pr == 0:
                if i == 0:
                    v_proj_half(1, 0)
                    v_proj_half(1, 1)
                elif i == 1:
                    v_proj(2)
                    v_proj(3)
                elif i < 14:
                    v_proj(i + 2)        # V arrives 2 k-tiles ahead
                if i in (0, 1):          # K(t0,n1) needed at k-tile 4
                    proj_half(wk_sb, kt_sb, bk_sb, 0, 1, i)
                if i in (3, 4):
                    proj_half(wk_sb, kt_sb, bk_sb, 0, 2, i - 3)
                if i in (6, 7):
                    proj_half(wk_sb, kt_sb, bk_sb, 0, 3, i - 6)
                if i in (9, 10):
                    proj_half(wk_sb, kt_sb, bk_sb, 1, 0, i - 9)
                if i in (11, 12):
                    proj_half(wq_sb, qt_sb, bq_sb, 1, 0, i - 11)
            elif n == 0 and pr == 1:
                if i in (0, 1):
                    proj_half(wk_sb, kt_sb, bk_sb, 1, 1, i)
                if i in (2, 3):
                    proj_half(wk_sb, kt_sb, bk_sb, 1, 2, i - 2)
                if i in (5, 6):
                    proj_half(wk_sb, kt_sb, bk_sb, 1, 3, i - 5)
                if i in (8, 9):
                    proj_half(wq_sb, qt_sb, bq_sb, 0, 1, i - 8)
            else:
                if i in (5, 6):
                    if pr == 1 and n + 1 < QC:
                        proj_half(wq_sb, qt_sb, bq_sb, 0, n + 1, i - 5)
                    elif pr == 0:
                        proj_half(wq_sb, qt_sb, bq_sb, 1, n, i - 5)
                if pr == 0 and i in (8, 10, 12, 14):
                    out_proj_m(n - 1, (i - 8) // 2)

        def norm_evac(n, pr, o_t, j, nq=512, qh=0):
            # fp32 evacuation: frees the PSUM o bank as fast as possible so
            # the next block's first PV matmul (stuck behind it in the PE
            # FIFO) is not stalled by the rest of the normalize chain
            ocp = nw.tile([DH + 1, 512], F32, tag="ocp", bufs=3,
                          padded_shape=[DH + 1, 512],
                          name=f"ocp_{n}_{pr}_{j}_{qh}")
            nc.vector.tensor_copy(ocp[:, 0:nq],
                                  o_t[j][:, qh * nq:qh * nq + nq])
            return ocp

        def norm_recip(n, pr, ocp, j, nq=512, qh=0):
            # denominator row 64 -> partition 0 (COPY-class op: the DVE
            # reshape front-end can cross partitions; custom DVE ops like
            # reciprocal cannot).  ocp may be the SBUF evacuation (qh=0,
            # full width) or the PSUM accumulator itself (tail, qh-sliced).
            off = 0 if ocp.shape[-1] == 512 and nq == 512 else qh * nq
            dsb = nw.tile([1, 512], F32, tag="dsb", padded_shape=[1, 512],
                          name=f"dsb_{n}_{pr}_{j}_{qh}")
            nc.vector.tensor_copy(dsb[:, 0:nq], ocp[DH:DH + 1, off:off + nq])
            r = nw.tile([1, 512], F32, tag="r", padded_shape=[1, 512],
                        name=f"r_{n}_{pr}_{j}_{qh}")
            nc.vector.reciprocal_approx_fast(r[:, 0:nq], dsb[:, 0:nq])
            # broadcast r across 64 partitions on GPSIMD
            rb = nw.tile([DH, 512], F32, tag="rb", padded_shape=[DH, 512],
                         name=f"rb_{n}_{pr}_{j}_{qh}")
            nc.gpsimd.partition_broadcast(rb[:, 0:nq], r[:, 0:nq])
            return rb

        def norm_mul(n, pr, ocp, rb, j, nq=512, qh=0):
            qs = slice(n * 512 + qh * nq, n * 512 + qh * nq + nq)
            off = 0 if ocp.shape[-1] == 512 and nq == 512 else qh * nq
            nc.vector.tensor_mul(out=on_sb[_ts(j, DH), pr, qs],
                                 in0=ocp[0:DH, off:off + nq],
                                 in1=rb[:, 0:nq])

        def norm_finish(n, pr, ocp, j, nq=512, qh=0):
            rb = norm_recip(n, pr, ocp, j, nq, qh)
            norm_mul(n, pr, ocp, rb, j, nq, qh)

        # ---- attention (64-row PE mode) + interleaved output projection ---
        # The PV matmuls run one tile LATE: each tile's PV is emitted after
        # the NEXT tile's scores+exp, so the ACT queue is never starved while
        # the PE drains PV pairs (matters most at block boundaries).
        pending = None           # (n, pr, i, o_t, et)

        def flush_pending():
            if pending is None:
                return
            pn, ppr, pi, po_t, pet = pending
            for j in range(2):
                nc.tensor.matmul(
                    po_t[j][:],
                    va_sb[:, pi, 2 * ppr + j, :],
                    pet[:, j, :],
                    start=(pi == 0), stop=(pi == KT - 1),
                )
            if pi == KT - 1:
                last = (pn == QC - 1) and (ppr == HG - 1)
                if not last:
                    # both evacuations first (fast PSUM release), then the
                    # lazy normalize chains
                    ocps = [norm_evac(pn, ppr, po_t, j) for j in range(2)]
                    for j in range(2):
                        norm_finish(pn, ppr, ocps[j], j)
                else:
                    # tail: no evacuation (the o banks never recycle at the
                    # end) — normalize straight out of PSUM, stage-ordered,
                    # out-DMAs spread across queues, half the fp16 casts on
                    # the idle ACT engine
                    rbs = {}
                    for qh in range(2):
                        for j in range(2):
                            sl = slice(qh * 256, qh * 256 + 256)
                            rbs[qh, j] = norm_recip(pn, ppr, po_t[j], j,
                                                    nq=256, qh=qh)
                    for qh in range(2):
                        for j in range(2):
                            norm_mul(pn, ppr, po_t[j], rbs[qh, j], j,
                                     nq=256, qh=qh)
                        for m in range(4):
                            out_proj_m(pn, m, nq=256, qh=qh,
                                       eng=(nc.sync, nc.scalar)[m % 2],
                                       cast_on_act=(m % 2 == 1))

        for n in range(QC):          # q chunk of 512
            for pr in range(HG):     # head pair (heads 2pr, 2pr+1)
                o_t = [ps.tile([DH + 1, 512], F32, tag="o", bufs=2,
                               name=f"o_{pr}_{n}_{j}")
                       for j in range(2)]
                for i in range(KT):
                    s = ps.tile([P, 2, 512], F32, tag="s", bufs=2,
                                name=f"s_{pr}_{n}_{i}")
                    for j in range(2):
                        nc.tensor.matmul(
                            s[:, j, :],
                            kt_sb[_ts(j, DH), pr, _ts(i, P)],
                            qt_sb[_ts(j, DH), pr, _ts(n, 512)],
                            start=True, stop=True,
                        )
                    et = ew.tile([P, 2, 512], BF16, tag="et", bufs=8)
                    nc.scalar.activation(et[:], s[:], Exp, scale=SCALE)
                    if n == 0 and pr == 0 and i == 0:
                        # va[0] is needed by the first PV; everything later
                        # comes via inject() after each tile
                        v_proj_half(0, 0)
                        v_proj_half(0, 1)
                    flush_pending()
                    pending = (n, pr, i, o_t, et)
                    inject(n, pr, i)
        flush_pending()              # last tile's PV + the tail


_CACHE = {}


def _get_nc():
    if "nc" not in _CACHE:
        # Bacc (not raw Bass): its compile() pipeline legalizes semaphore
        # waits (TRN2 allows at most one wait per instruction).
        nc = bacc.Bacc(None, target_bir_lowering=False)
        with tile.TileContext(nc) as tc:
            _body(tc)
        nc.finalize()
        _CACHE["nc"] = nc
    return _CACHE["nc"]


def make_in_maps(x, Wq, bq, Wk, bk, Wv, bv, Wo):
    bf = ml_dtypes.bfloat16
    in_maps = []
    for c in range(8):
        b, g = c // 2, c % 2
        es = slice(g * EG, (g + 1) * EG)
        bpk = np.zeros((P, 4), np.float32)
        bpk[:, 0] = np.asarray(bq)[es][0:P]
        bpk[:, 1] = np.asarray(bq)[es][P:2 * P]
        bpk[:, 2] = np.asarray(bk)[es][0:P]
        bpk[:, 3] = np.asarray(bk)[es][P:2 * P]
        in_maps.append({
            "xT": np.ascontiguousarray(np.asarray(x)[b].T).astype(bf),
            "wqT": np.ascontiguousarray(np.asarray(Wq)[es, :].T).astype(bf),
            "wkT": np.ascontiguousarray(np.asarray(Wk)[es, :].T).astype(bf),
            "wvT": np.ascontiguousarray(np.asarray(Wv)[es, :].T).astype(bf),
            "woT": np.ascontiguousarray(np.asarray(Wo)[:, es].T).astype(bf),
            "bpk": bpk,
        })
    return in_maps


def gather_out(results, bo, bv, Wo):
    # device partials exclude the V bias: (O/d) @ Wo.  bv passes through the
    # attention untouched (softmax rows sum to 1), so its contribution is the
    # constant vector bv @ Wo.T, added here with bo.
    const = (np.asarray(bo, np.float64)
             + np.asarray(bv, np.float64) @ np.asarray(Wo, np.float64).T
             ).astype(np.float32)
    out = np.empty((B, L, D), np.float32)
    for b in range(B):
        out[b] = (results[2 * b]["outT"].astype(np.float32).T
                  + results[2 * b + 1]["outT"].astype(np.float32).T
                  + const[None, :])
    return out


def kernel(x, Wq, bq, Wk, bk, Wv, bv, Wo, bo, **kwargs):
    nc = _get_nc()
    in_maps = make_in_maps(x, Wq, bq, Wk, bk, Wv, bv, Wo)
    res = run_bass_kernel_spmd(nc, in_maps, list(range(8)))
    return gather_out(res.results, bo, bv, Wo)


# revision 33
# speedup vs baseline: 1.0029x; 1.0029x over previous
"""Multi-head self-attention (b=4, L=2048, d=512, h=8) on 8 trn2 cores.

Sharding: data-parallel over batch (4) x tensor-parallel over heads (2 groups
of 4).  Core c handles batch c//2, heads [4*(c%2), 4*(c%2)+4).  Each core
returns a partial output (row-parallel Wo); the host sums the two partials per
batch and adds bo.

Device-side layout (all "transposed", so no on-device transposes are needed):
  xT   [512 d, 2048 q]   (host passes x[b].T, bf16)
  Q^T  [256 hd, 2048 q]  = WqT.T @ xT   (lhsT = WqT[d, hd] natural)
  K^T  same
  V    [2048 k, 256 hd]  (lhsT = xT chunks)  + ones column per head -> V_aug
  S^T  [128 k-tile, 512 q] = K^T_h.T-contraction(d_head=64)  -> PE 64-row mode,
       two heads of a pair run on independent half-arrays (T0 / T8)
  E^T  = exp(S^T * scale)  on ACT, one 1024-wide call per (k-tile, head-pair)
  O^T_aug [65, 512q] += V_aug_h.T @ E^T_h, giving O = P@V un-normalized plus
       the softmax denominator d[q] in row 64 (from the ones column).
  normalize: O = O[0:64] * broadcast(1/d)  (DVE; bv is folded into the host
       gather since softmax rows sum to 1: partial includes no bias, host
       adds bo + bv @ Wo.T)
  outT [512 e, 2048 q] = WoT.T @ O_norm^T  (fp16 partial, host sums)

Scheduling (the steady state runs the ACT engine ~98% busy at ~1.12us per
k-tile with the PE saturated underneath):
  - input DMAs split across the sync/scalar/gpsimd HW queues, ordered so the
    critical path (wk, wq, x chunk 0) lands first; x0 is split by
    contraction halves so the first K/Q projection halves start early
  - ~5us of tiny PE warmup matmuls bridge the DMA wait so the HAM clock
    gate is at 2.4 GHz when the first projection issues
  - all filler work (V/K/Q projections, previous chunk's output projection)
    is emitted AFTER each tile's scores/exp/PV so the FIFO engine queues
    serve the critical loop first; projection units are emitted in halves
  - each tile's PV pair is emitted one tile late (after the next tile's
    scores+exp) so ACT never waits on PE PV drain at block boundaries
  - PSUM o accumulators are evacuated to SBUF immediately at block end
    (both heads first), the 1/d normalize chain then runs lazily off the
    critical path
  - the final block's normalize + output projection run in q-halves with
    casts split between DVE and the idle ACT engine and out-DMAs spread
    over two queues to compress the tail
"""

import numpy as np
import ml_dtypes

import concourse.bass as bass
import concourse.bacc as bacc
import concourse.tile as tile
import concourse.mybir as mybir
from concourse.bass_utils import run_bass_kernel_spmd

F32 = mybir.dt.float32
F16 = mybir.dt.float16
BF16 = mybir.dt.bfloat16

B, L, D = 4, 2048, 512
NH, DH = 8, 64
HG = 2                 # head groups (tensor parallel)
GH = NH // HG          # 4 heads per group
EG = GH * DH           # 256 columns per group
SCALE = 1.0 / float(np.sqrt(DH))
P = 128
KT = L // P            # 16 k-tiles
QC = L // 512          # 4 q-chunks of 512
DC = D // P            # 4 d-chunks

_ts = bass.ts


def _body(tc):
    nc = tc.nc
    xT = nc.dram_tensor("xT", [D, L], BF16, kind="ExternalInput")
    wqT = nc.dram_tensor("wqT", [D, EG], BF16, kind="ExternalInput")
    wkT = nc.dram_tensor("wkT", [D, EG], BF16, kind="ExternalInput")
    wvT = nc.dram_tensor("wvT", [D, EG], BF16, kind="ExternalInput")
    woT = nc.dram_tensor("woT", [EG, D], BF16, kind="ExternalInput")
    # packed biases: col 0,1 = bq (t0,t1); col 2,3 = bk.  (bv is folded into
    # the host-side gather: (O/d + bv) @ Wo = (O/d) @ Wo + bv @ Wo, and
    # bv @ Wo.T is a constant [512] vector added with bo.)
    bpk = nc.dram_tensor("bpk", [P, 4], F32, kind="ExternalInput")
    outT = nc.dram_tensor("outT", [D, L], F16, kind="ExternalOutput")

    add = mybir.AluOpType.add
    Exp = mybir.ActivationFunctionType.Exp

    with (
        tc.tile_pool(name="const", bufs=1) as const,
        # one PSUM pool for the whole kernel so the phases can overlap:
        #   tag "s": score tiles [128,2,512] (2 banks) x2  -> 4 banks
        #   tag "o": PV accumulators [65,512] x2           -> 2 banks
        #   tag "u": proj / out-proj psum [128,512] x2     -> 2 banks
        tc.tile_pool(name="ps", bufs=2, space="PSUM") as ps,
        tc.tile_pool(name="ew", bufs=3) as ew,
        tc.tile_pool(name="nw", bufs=3) as nw,
        tc.tile_pool(name="ow", bufs=4) as ow,
    ):
        # ---- load inputs; order == Sync-queue issue order == transfer order.
        # Critical path to the first exp: wk(t0), x(n0), wq(t0), biases.
        wq_sb = const.tile([P, DC, EG], BF16)
        wk_sb = const.tile([P, DC, EG], BF16)
        wv_sb = const.tile([P, DC, EG], BF16)
        xT_sb = const.tile([P, DC, L], BF16)
        wo_sb = const.tile([P, HG, D], BF16)
        bias_sb = const.tile([P, 4], F32)

        def w_load(eng, w_sb, w_dr, t):
            eng.dma_start(
                out=w_sb[:, :, _ts(t, P)],
                in_=w_dr[:, _ts(t, P)].rearrange("(c p) e -> p c e", p=P))

        def x_load(eng, n, cs=slice(0, DC)):
            eng.dma_start(
                out=xT_sb[:, cs, _ts(n, 512)],
                in_=xT[:, _ts(n, 512)].rearrange("(c p) k -> p c k", p=P)[:, cs, :])

        # input loads spread across three idle engine DMA queues so the
        # descriptors issue (and transfers start) in parallel; critical path
        # to the first exp is wk, wq, x0(c=0,1)
        wv_r = wvT[:, :].rearrange("(c p) e -> p c e", p=P)
        w_load(nc.sync, wk_sb, wkT, 0)
        w_load(nc.scalar, wq_sb, wqT, 0)
        nc.gpsimd.dma_start(out=bias_sb[:], in_=bpk[:, :])
        # x0 split across both queues so its halves transfer in parallel
        x_load(nc.sync, 0, slice(0, 1))
        x_load(nc.scalar, 0, slice(1, 2))
        x_load(nc.sync, 0, slice(2, 3))
        x_load(nc.scalar, 0, slice(3, 4))
        nc.sync.dma_start(out=wv_sb[:, 0:2, :], in_=wv_r[:, 0:2, :])
        nc.scalar.dma_start(out=wv_sb[:, 2:4, :], in_=wv_r[:, 2:4, :])
        x_load(nc.sync, 1)
        x_load(nc.scalar, 2)
        w_load(nc.sync, wk_sb, wkT, 1)
        w_load(nc.scalar, wq_sb, wqT, 1)
        x_load(nc.sync, 3)
        nc.scalar.dma_start(
            out=wo_sb[:], in_=woT[:, :].rearrange("(c p) e -> p c e", p=P))

        bq_sb = bias_sb[:, 0:2]
        bk_sb = bias_sb[:, 2:4]

        # persistent activations
        qt_sb = const.tile([P, HG, L], BF16)     # Q^T  rows: head 2t + (r//64)
        kt_sb = const.tile([P, HG, L], BF16)     # K^T
        va_sb = const.tile([P, KT, GH, DH + 1], BF16)  # V + ones col, per k-tile
        on_sb = const.tile([P, HG, L], BF16)     # normalized O^T (attn output)

        nc.vector.memset(va_sb[:, :, :, DH:DH + 1], 1.0)

        # PE warmup: ~4.3us of full-array matmuls while the input DMAs are
        # in flight, so the HAM clock gate reaches 2.4 GHz before the first
        # projection (cold PE runs at 1.2 GHz; the HAM watches array
        # activity, so the warmup must light up the whole 128x128 array)
        warm = const.tile([P, 512], BF16)
        nc.vector.memset(warm[:], 0.0)
        wps = ps.tile([P, 512], F32, tag="u", name="warm_ps")
        for _ in range(60):
            nc.tensor.matmul(wps[:, 0:96], warm[:, 0:P], warm[:, 0:96],
                             start=True, stop=True)

        # ---- projections (128-row PE mode), emitted just-in-time ----------
        # Split into halves so an injected unit never monopolizes the PE
        # queue ahead of the next score tile.
        proj_ps = {}

        def proj_half(w_sb, dst, b_sb, t, n, half, on_act=False):
            key = (id(w_sb), t, n)
            if half == 0:
                proj_ps[key] = ps.tile([P, 512], F32, tag="u",
                                       name=f"p_ps_{t}_{n}")
            psq = proj_ps[key]
            for c in (0, 1) if half == 0 else (2, 3):
                nc.tensor.matmul(
                    psq[:],
                    w_sb[:, c, _ts(t, P)],
                    xT_sb[:, c, _ts(n, 512)],
                    start=(c == 0), stop=(c == DC - 1),
                )
            if half == 1:
                if on_act:
                    nc.scalar.add(dst[:, t, _ts(n, 512)], psq[:],
                                  b_sb[:, t:t + 1])
                else:
                    nc.vector.tensor_scalar(
                        out=dst[:, t, _ts(n, 512)], in0=psq[:],
                        scalar1=b_sb[:, t:t + 1], scalar2=None, op0=add,
                    )

        def proj_unit(w_sb, dst, b_sb, t, n):
            proj_half(w_sb, dst, b_sb, t, n, 0)
            proj_half(w_sb, dst, b_sb, t, n, 1)

        def v_proj(i):
            psv = ps.tile([P, EG], F32, tag="u", name=f"v_ps_{i}")
            for c in range(DC):
                nc.tensor.matmul(
                    psv[:],
                    xT_sb[:, c, _ts(i, P)],
                    wv_sb[:, c, :],
                    start=(c == 0), stop=(c == DC - 1),
                )
            nc.vector.tensor_copy(
                out=va_sb[:, i, :, 0:DH],
                in_=psv[:].rearrange("p (h d) -> p h d", d=DH),
            )

        def v_proj_half(i, half):
            key = ("v", i)
            if half == 0:
                proj_ps[key] = ps.tile([P, EG], F32, tag="u",
                                       name=f"v_ps_{i}")
            psv = proj_ps[key]
            for c in (0, 1) if half == 0 else (2, 3):
                nc.tensor.matmul(
                    psv[:],
                    xT_sb[:, c, _ts(i, P)],
                    wv_sb[:, c, :],
                    start=(c == 0), stop=(c == DC - 1),
                )
            if half == 1:
                nc.vector.tensor_copy(
                    out=va_sb[:, i, :, 0:DH],
                    in_=psv[:].rearrange("p (h d) -> p h d", d=DH),
                )

        def out_proj_m(n, m, nq=512, qh=0, eng=None, cast_on_act=False):
            eng = eng or nc.sync
            # one e-tile of the output projection for q-chunk n
            pso = ps.tile([P, 512], F32, tag="u", name=f"o_ps_{m}_{n}_{qh}",
                          padded_shape=[P, 512])
            qs = slice(n * 512 + qh * nq, n * 512 + qh * nq + nq)
            for c in range(HG):
                nc.tensor.matmul(
                    pso[:, 0:nq],
                    wo_sb[:, c, _ts(m, P)],
                    on_sb[:, c, qs],
                    start=(c == 0), stop=(c == HG - 1),
                )
            st = ow.tile([P, 512], F16, tag="o_st", padded_shape=[P, 512],
                         name=f"o_st_{m}_{n}_{qh}")
            if cast_on_act:
                nc.scalar.copy(st[:, 0:nq], pso[:, 0:nq])
            else:
                nc.vector.tensor_copy(st[:, 0:nq], pso[:, 0:nq])
            eng.dma_start(
                out=outT[:, :][_ts(m, P), qs], in_=st[:, 0:nq],
            )

        # Upfront: only what the first k-tiles of block (0,0) need.  Emitted
        # as interleaved halves so the K half on x(c=0,1) starts while
        # x(c=2,3) is still in flight.
        proj_half(wk_sb, kt_sb, bk_sb, 0, 0, 0)
        proj_half(wq_sb, qt_sb, bq_sb, 0, 0, 0)
        proj_half(wk_sb, kt_sb, bk_sb, 0, 0, 1, on_act=True)
        proj_half(wq_sb, qt_sb, bq_sb, 0, 0, 1)

        def inject(n, pr, i):
            """Emit filler work (projections, previous chunk's output
            projection) AFTER tile i's attention ops so the FIFO PE queue
            serves scores/PV first and the filler soaks up ACT-bound slack."""
            if n == 0 and pr == 0:
                if i == 0:
                    v_proj_half(1, 0)
                    v_proj_half(1, 1)
                elif i == 1:
                    v_proj(2)
                    v_proj(3)
                elif i < 14:
                    v_proj(i + 2)        # V arrives 2 k-tiles ahead
                if i in (0, 1):          # K(t0,n1) needed at k-tile 4
                    proj_half(wk_sb, kt_sb, bk_sb, 0, 1, i)
                if i in (3, 4):
                    proj_half(wk_sb, kt_sb, bk_sb, 0, 2, i - 3)
                if i in (6, 7):
                    proj_half(wk_sb, kt_sb, bk_sb, 0, 3, i - 6)
                if i in (9, 10):
                    proj_half(wk_sb, kt_sb, bk_sb, 1, 0, i - 9)
                if i in (11, 12):
                    proj_half(wq_sb, qt_sb, bq_sb, 1, 0, i - 11)
            elif n == 0 and pr == 1:
                if i in (0, 1):
                    proj_half(wk_sb, kt_sb, bk_sb, 1, 1, i)
                if i in (2, 3):
                    proj_half(wk_sb, kt_sb, bk_sb, 1, 2, i - 2)
                if i in (5, 6):
                    proj_half(wk_sb, kt_sb, bk_sb, 1, 3, i - 5)
                if i in (8, 9):
                    proj_half(wq_sb, qt_sb, bq_sb, 0, 1, i - 8)
            else:
                if i in (5, 6):
                    if pr == 1 and n + 1 < QC:
                        proj_half(wq_sb, qt_sb, bq_sb, 0, n + 1, i - 5)
                    elif pr == 0:
                        proj_half(wq_sb, qt_sb, bq_sb, 1, n, i - 5)
                if pr == 0 and i in (8, 10, 12, 14):
                    out_proj_m(n - 1, (i - 8) // 2)

        def norm_evac(n, pr, o_t, j, nq=512, qh=0):
            # fp32 evacuation: frees the PSUM o bank as fast as possible so
            # the next block's first PV matmul (stuck behind it in the PE
            # FIFO) is not stalled by the rest of the normalize chain
            ocp = nw.tile([DH + 1, 512], F32, tag="ocp", bufs=3,
                          padded_shape=[DH + 1, 512],
                          name=f"ocp_{n}_{pr}_{j}_{qh}")
            nc.vector.tensor_copy(ocp[:, 0:nq],
                                  o_t[j][:, qh * nq:qh * nq + nq])
            return ocp

        def norm_recip(n, pr, ocp, j, nq=512, qh=0):
            # denominator row 64 -> partition 0 (COPY-class op: the DVE
            # reshape front-end can cross partitions; custom DVE ops like
            # reciprocal cannot).  ocp may be the SBUF evacuation (qh=0,
            # full width) or the PSUM accumulator itself (tail, qh-sliced).
            off = 0 if ocp.shape[-1] == 512 and nq == 512 else qh * nq
            dsb = nw.tile([1, 512], F32, tag="dsb", padded_shape=[1, 512],
                          name=f"dsb_{n}_{pr}_{j}_{qh}")
            nc.vector.tensor_copy(dsb[:, 0:nq], ocp[DH:DH + 1, off:off + nq])
            r = nw.tile([1, 512], F32, tag="r", padded_shape=[1, 512],
                        name=f"r_{n}_{pr}_{j}_{qh}")
            nc.vector.reciprocal_approx_fast(r[:, 0:nq], dsb[:, 0:nq])
            # broadcast r across 64 partitions on GPSIMD
            rb = nw.tile([DH, 512], F32, tag="rb", padded_shape=[DH, 512],
                         name=f"rb_{n}_{pr}_{j}_{qh}")
            nc.gpsimd.partition_broadcast(rb[:, 0:nq], r[:, 0:nq])
            return rb

        def norm_mul(n, pr, ocp, rb, j, nq=512, qh=0):
            qs = slice(n * 512 + qh * nq, n * 512 + qh * nq + nq)
            off = 0 if ocp.shape[-1] == 512 and nq == 512 else qh * nq
            nc.vector.tensor_mul(out=on_sb[_ts(j, DH), pr, qs],
                                 in0=ocp[0:DH, off:off + nq],
                                 in1=rb[:, 0:nq])

        def norm_finish(n, pr, ocp, j, nq=512, qh=0):
            rb = norm_recip(n, pr, ocp, j, nq, qh)
            norm_mul(n, pr, ocp, rb, j, nq, qh)

        # ---- attention (64-row PE mode) + interleaved output projection ---
        # The PV matmuls run one tile LATE: each tile's PV is emitted after
        # the NEXT tile's scores+exp, so the ACT queue is never starved while
        # the PE drains PV pairs (matters most at block boundaries).
        pending = None           # (n, pr, i, o_t, et)

        def flush_pending():
            if pending is None:
                return
            pn, ppr, pi, po_t, pet = pending
            for j in range(2):
                nc.tensor.matmul(
                    po_t[j][:],
                    va_sb[:, pi, 2 * ppr + j, :],
                    pet[:, j, :],
                    start=(pi == 0), stop=(pi == KT - 1),
                )
            if pi == KT - 1:
                last = (pn == QC - 1) and (ppr == HG - 1)
                if not last:
                    # both evacuations first (fast PSUM release), then the
                    # lazy normalize chains
                    ocps = [norm_evac(pn, ppr, po_t, j) for j in range(2)]
                    for j in range(2):
                        norm_finish(pn, ppr, ocps[j], j)
                else:
                    # tail: no evacuation (the o banks never recycle at the
                    # end) — normalize straight out of PSUM, one combined
                    # broadcast per q-half, out-DMAs spread across queues,
                    # half the fp16 casts on the idle ACT engine
                    rbs = {}
                    for qh in range(2):
                        r2 = nw.tile([1, 512], F32, tag="r",
                                     padded_shape=[1, 512],
                                     name=f"r2_{qh}")
                        for j in range(2):
                            sl = slice(qh * 256, qh * 256 + 256)
                            dsb = nw.tile([1, 512], F32, tag="dsb",
                                          padded_shape=[1, 512],
                                          name=f"dsbt_{j}_{qh}")
                            nc.vector.tensor_copy(dsb[:, 0:256],
                                                  po_t[j][DH:DH + 1, sl])
                            nc.vector.reciprocal_approx_fast(
                                r2[:, j * 256:j * 256 + 256], dsb[:, 0:256])
                        rb2 = nw.tile([DH, 512], F32, tag="rb",
                                      padded_shape=[DH, 512],
                                      name=f"rb2_{qh}")
                        nc.gpsimd.partition_broadcast(rb2[:], r2[:])
                        rbs[qh] = rb2
                    for qh in range(2):
                        for j in range(2):
                            sl = slice(qh * 256, qh * 256 + 256)
                            qs = slice(pn * 512 + qh * 256,
                                       pn * 512 + qh * 256 + 256)
                            nc.vector.tensor_mul(
                                out=on_sb[_ts(j, DH), ppr, qs],
                                in0=po_t[j][0:DH, sl],
                                in1=rbs[qh][:, j * 256:j * 256 + 256])
                        for m in range(4):
                            out_proj_m(pn, m, nq=256, qh=qh,
                                       eng=(nc.sync, nc.scalar)[m % 2],
                                       cast_on_act=(m % 2 == 1))

        for n in range(QC):          # q chunk of 512
            for pr in range(HG):     # head pair (heads 2pr, 2pr+1)
                o_t = [ps.tile([DH + 1, 512], F32, tag="o", bufs=2,
                               name=f"o_{pr}_{n}_{j}")
                       for j in range(2)]
                for i in range(KT):
                    s = ps.tile([P, 2, 512], F32, tag="s", bufs=2,
                                name=f"s_{pr}_{n}_{i}")
                    for j in range(2):
                        nc.tensor.matmul(
                            s[:, j, :],
                            kt_sb[_ts(j, DH), pr, _ts(i, P)],
                            qt_sb[_ts(j, DH), pr, _ts(n, 512)],
                            start=True, stop=True,
                        )
                    et = ew.tile([P, 2, 512], BF16, tag="et", bufs=8)
                    nc.scalar.activation(et[:], s[:], Exp, scale=SCALE)
                    if n == 0 and pr == 0 and i == 0:
                        # va[0] is needed by the first PV; everything later
                        # comes via inject() after each tile
                        v_proj_half(0, 0)
                        v_proj_half(0, 1)
                    flush_pending()
                    pending = (n, pr, i, o_t, et)
                    inject(n, pr, i)
        flush_pending()              # last tile's PV + the tail


_CACHE = {}


def _get_nc():
    if "nc" not in _CACHE:
        # Bacc (not raw Bass): its compile() pipeline legalizes semaphore
        # waits (TRN2 allows at most one wait per instruction).
        nc = bacc.Bacc(None, target_bir_lowering=False)
        with tile.TileContext(nc) as tc:
            _body(tc)
        nc.finalize()
        _CACHE["nc"] = nc
    return _CACHE["nc"]


def make_in_maps(x, Wq, bq, Wk, bk, Wv, bv, Wo):
    bf = ml_dtypes.bfloat16
    in_maps = []
    for c in range(8):
        b, g = c // 2, c % 2
        es = slice(g * EG, (g + 1) * EG)
        bpk = np.zeros((P, 4), np.float32)
        bpk[:, 0] = np.asarray(bq)[es][0:P]
        bpk[:, 1] = np.asarray(bq)[es][P:2 * P]
        bpk[:, 2] = np.asarray(bk)[es][0:P]
        bpk[:, 3] = np.asarray(bk)[es][P:2 * P]
        in_maps.append({
            "xT": np.ascontiguousarray(np.asarray(x)[b].T).astype(bf),
            "wqT": np.ascontiguousarray(np.asarray(Wq)[es, :].T).astype(bf),
            "wkT": np.ascontiguousarray(np.asarray(Wk)[es, :].T).astype(bf),
            "wvT": np.ascontiguousarray(np.asarray(Wv)[es, :].T).astype(bf),
            "woT": np.ascontiguousarray(np.asarray(Wo)[:, es].T).astype(bf),
            "bpk": bpk,
        })
    return in_maps


def gather_out(results, bo, bv, Wo):
    # device partials exclude the V bias: (O/d) @ Wo.  bv passes through the
    # attention untouched (softmax rows sum to 1), so its contribution is the
    # constant vector bv @ Wo.T, added here with bo.
    const = (np.asarray(bo, np.float64)
             + np.asarray(bv, np.float64) @ np.asarray(Wo, np.float64).T
             ).astype(np.float32)
    out = np.empty((B, L, D), np.float32)
    for b in range(B):
        out[b] = (results[2 * b]["outT"].astype(np.float32).T
                  + results[2 * b + 1]["outT"].astype(np.float32).T
                  + const[None, :])
    return out


def kernel(x, Wq, bq, Wk, bk, Wv, bv, Wo, bo, **kwargs):
    nc = _get_nc()
    in_maps = make_in_maps(x, Wq, bq, Wk, bk, Wv, bv, Wo)
    res = run_bass_kernel_spmd(nc, in_maps, list(range(8)))
    return gather_out(res.results, bo, bv, Wo)


# revision 34
# speedup vs baseline: 1.0041x; 1.0012x over previous
"""Multi-head self-attention (b=4, L=2048, d=512, h=8) on 8 trn2 cores.

Sharding: data-parallel over batch (4) x tensor-parallel over heads (2 groups
of 4).  Core c handles batch c//2, heads [4*(c%2), 4*(c%2)+4).  Each core
returns a partial output (row-parallel Wo); the host sums the two partials per
batch and adds bo.

Device-side layout (all "transposed", so no on-device transposes are needed):
  xT   [512 d, 2048 q]   (host passes x[b].T, bf16)
  Q^T  [256 hd, 2048 q]  = WqT.T @ xT   (lhsT = WqT[d, hd] natural)
  K^T  same
  V    [2048 k, 256 hd]  (lhsT = xT chunks)  + ones column per head -> V_aug
  S^T  [128 k-tile, 512 q] = K^T_h.T-contraction(d_head=64)  -> PE 64-row mode,
       two heads of a pair run on independent half-arrays (T0 / T8)
  E^T  = exp(S^T * scale)  on ACT, one 1024-wide call per (k-tile, head-pair)
  O^T_aug [65, 512q] += V_aug_h.T @ E^T_h, giving O = P@V un-normalized plus
       the softmax denominator d[q] in row 64 (from the ones column).
  normalize: O = O[0:64] * broadcast(1/d)  (DVE; bv is folded into the host
       gather since softmax rows sum to 1: partial includes no bias, host
       adds bo + bv @ Wo.T)
  outT [512 e, 2048 q] = WoT.T @ O_norm^T  (fp16 partial, host sums)

Scheduling (the steady state runs the ACT engine ~98% busy at ~1.12us per
k-tile with the PE saturated underneath):
  - input DMAs split across the sync/scalar/gpsimd HW queues, ordered so the
    critical path (wk, wq, x chunk 0) lands first; x0 is split by
    contraction halves so the first K/Q projection halves start early
  - ~5us of tiny PE warmup matmuls bridge the DMA wait so the HAM clock
    gate is at 2.4 GHz when the first projection issues
  - all filler work (V/K/Q projections, previous chunk's output projection)
    is emitted AFTER each tile's scores/exp/PV so the FIFO engine queues
    serve the critical loop first; projection units are emitted in halves
  - each tile's PV pair is emitted one tile late (after the next tile's
    scores+exp) so ACT never waits on PE PV drain at block boundaries
  - PSUM o accumulators are evacuated to SBUF immediately at block end
    (both heads first), the 1/d normalize chain then runs lazily off the
    critical path
  - the final block's normalize + output projection run in q-halves with
    casts split between DVE and the idle ACT engine and out-DMAs spread
    over two queues to compress the tail
"""

import numpy as np
import ml_dtypes

import concourse.bass as bass
import concourse.bacc as bacc
import concourse.tile as tile
import concourse.mybir as mybir
from concourse.bass_utils import run_bass_kernel_spmd

F32 = mybir.dt.float32
F16 = mybir.dt.float16
BF16 = mybir.dt.bfloat16

B, L, D = 4, 2048, 512
NH, DH = 8, 64
HG = 2                 # head groups (tensor parallel)
GH = NH // HG          # 4 heads per group
EG = GH * DH           # 256 columns per group
SCALE = 1.0 / float(np.sqrt(DH))
P = 128
KT = L // P            # 16 k-tiles
QC = L // 512          # 4 q-chunks of 512
DC = D // P            # 4 d-chunks

_ts = bass.ts


def _body(tc):
    nc = tc.nc
    xT = nc.dram_tensor("xT", [D, L], BF16, kind="ExternalInput")
    wqT = nc.dram_tensor("wqT", [D, EG], BF16, kind="ExternalInput")
    wkT = nc.dram_tensor("wkT", [D, EG], BF16, kind="ExternalInput")
    wvT = nc.dram_tensor("wvT", [D, EG], BF16, kind="ExternalInput")
    woT = nc.dram_tensor("woT", [EG, D], BF16, kind="ExternalInput")
    # packed biases: col 0,1 = bq (t0,t1); col 2,3 = bk.  (bv is folded into
    # the host-side gather: (O/d + bv) @ Wo = (O/d) @ Wo + bv @ Wo, and
    # bv @ Wo.T is a constant [512] vector added with bo.)
    bpk = nc.dram_tensor("bpk", [P, 4], F32, kind="ExternalInput")
    outT = nc.dram_tensor("outT", [D, L], F16, kind="ExternalOutput")

    add = mybir.AluOpType.add
    Exp = mybir.ActivationFunctionType.Exp

    with (
        tc.tile_pool(name="const", bufs=1) as const,
        # one PSUM pool for the whole kernel so the phases can overlap:
        #   tag "s": score tiles [128,2,512] (2 banks) x2  -> 4 banks
        #   tag "o": PV accumulators [65,512] x2           -> 2 banks
        #   tag "u": proj / out-proj psum [128,512] x2     -> 2 banks
        tc.tile_pool(name="ps", bufs=2, space="PSUM") as ps,
        tc.tile_pool(name="ew", bufs=3) as ew,
        tc.tile_pool(name="nw", bufs=3) as nw,
        tc.tile_pool(name="ow", bufs=4) as ow,
    ):
        # ---- load inputs; order == Sync-queue issue order == transfer order.
        # Critical path to the first exp: wk(t0), x(n0), wq(t0), biases.
        wq_sb = const.tile([P, DC, EG], BF16)
        wk_sb = const.tile([P, DC, EG], BF16)
        wv_sb = const.tile([P, DC, EG], BF16)
        xT_sb = const.tile([P, DC, L], BF16)
        wo_sb = const.tile([P, HG, D], BF16)
        bias_sb = const.tile([P, 4], F32)

        def w_load(eng, w_sb, w_dr, t):
            eng.dma_start(
                out=w_sb[:, :, _ts(t, P)],
                in_=w_dr[:, _ts(t, P)].rearrange("(c p) e -> p c e", p=P))

        def x_load(eng, n, cs=slice(0, DC)):
            eng.dma_start(
                out=xT_sb[:, cs, _ts(n, 512)],
                in_=xT[:, _ts(n, 512)].rearrange("(c p) k -> p c k", p=P)[:, cs, :])

        # input loads spread across three idle engine DMA queues so the
        # descriptors issue (and transfers start) in parallel; critical path
        # to the first exp is wk, wq, x0(c=0,1)
        wv_r = wvT[:, :].rearrange("(c p) e -> p c e", p=P)
        w_load(nc.sync, wk_sb, wkT, 0)
        w_load(nc.scalar, wq_sb, wqT, 0)
        nc.gpsimd.dma_start(out=bias_sb[:], in_=bpk[:, :])
        # x0 split across both queues so its halves transfer in parallel
        x_load(nc.sync, 0, slice(0, 1))
        x_load(nc.scalar, 0, slice(1, 2))
        x_load(nc.sync, 0, slice(2, 3))
        x_load(nc.scalar, 0, slice(3, 4))
        nc.sync.dma_start(out=wv_sb[:, 0:2, :], in_=wv_r[:, 0:2, :])
        nc.scalar.dma_start(out=wv_sb[:, 2:4, :], in_=wv_r[:, 2:4, :])
        x_load(nc.sync, 1)
        x_load(nc.scalar, 2)
        w_load(nc.sync, wk_sb, wkT, 1)
        w_load(nc.scalar, wq_sb, wqT, 1)
        x_load(nc.sync, 3)
        nc.scalar.dma_start(
            out=wo_sb[:], in_=woT[:, :].rearrange("(c p) e -> p c e", p=P))

        bq_sb = bias_sb[:, 0:2]
        bk_sb = bias_sb[:, 2:4]

        # persistent activations
        qt_sb = const.tile([P, HG, L], BF16)     # Q^T  rows: head 2t + (r//64)
        kt_sb = const.tile([P, HG, L], BF16)     # K^T
        va_sb = const.tile([P, KT, GH, DH + 1], BF16)  # V + ones col, per k-tile
        on_sb = const.tile([P, HG, L], BF16)     # normalized O^T (attn output)

        nc.vector.memset(va_sb[:, :, :, DH:DH + 1], 1.0)

        # PE warmup: ~4.3us of full-array matmuls while the input DMAs are
        # in flight, so the HAM clock gate reaches 2.4 GHz before the first
        # projection (cold PE runs at 1.2 GHz; the HAM watches array
        # activity, so the warmup must light up the whole 128x128 array)
        warm = const.tile([P, 512], BF16)
        nc.vector.memset(warm[:], 0.0)
        wps = ps.tile([P, 512], F32, tag="u", name="warm_ps")
        for _ in range(60):
            nc.tensor.matmul(wps[:, 0:96], warm[:, 0:P], warm[:, 0:96],
                             start=True, stop=True)

        # ---- projections (128-row PE mode), emitted just-in-time ----------
        # Split into halves so an injected unit never monopolizes the PE
        # queue ahead of the next score tile.
        proj_ps = {}

        def proj_half(w_sb, dst, b_sb, t, n, half, on_act=False):
            key = (id(w_sb), t, n)
            if half == 0:
                proj_ps[key] = ps.tile([P, 512], F32, tag="u",
                                       name=f"p_ps_{t}_{n}")
            psq = proj_ps[key]
            for c in (0, 1) if half == 0 else (2, 3):
                nc.tensor.matmul(
                    psq[:],
                    w_sb[:, c, _ts(t, P)],
                    xT_sb[:, c, _ts(n, 512)],
                    start=(c == 0), stop=(c == DC - 1),
                )
            if half == 1:
                if on_act:
                    nc.scalar.add(dst[:, t, _ts(n, 512)], psq[:],
                                  b_sb[:, t:t + 1])
                else:
                    nc.vector.tensor_scalar(
                        out=dst[:, t, _ts(n, 512)], in0=psq[:],
                        scalar1=b_sb[:, t:t + 1], scalar2=None, op0=add,
                    )

        def proj_unit(w_sb, dst, b_sb, t, n):
            proj_half(w_sb, dst, b_sb, t, n, 0)
            proj_half(w_sb, dst, b_sb, t, n, 1)

        def v_proj(i):
            psv = ps.tile([P, EG], F32, tag="u", name=f"v_ps_{i}")
            for c in range(DC):
                nc.tensor.matmul(
                    psv[:],
                    xT_sb[:, c, _ts(i, P)],
                    wv_sb[:, c, :],
                    start=(c == 0), stop=(c == DC - 1),
                )
            nc.vector.tensor_copy(
                out=va_sb[:, i, :, 0:DH],
                in_=psv[:].rearrange("p (h d) -> p h d", d=DH),
            )

        def v_proj_half(i, half):
            key = ("v", i)
            if half == 0:
                proj_ps[key] = ps.tile([P, EG], F32, tag="u",
                                       name=f"v_ps_{i}")
            psv = proj_ps[key]
            for c in (0, 1) if half == 0 else (2, 3):
                nc.tensor.matmul(
                    psv[:],
                    xT_sb[:, c, _ts(i, P)],
                    wv_sb[:, c, :],
                    start=(c == 0), stop=(c == DC - 1),
                )
            if half == 1:
                nc.vector.tensor_copy(
                    out=va_sb[:, i, :, 0:DH],
                    in_=psv[:].rearrange("p (h d) -> p h d", d=DH),
                )

        def out_proj_m(n, m, nq=512, qh=0, eng=None, cast_on_act=False):
            eng = eng or nc.sync
            # one e-tile of the output projection for q-chunk n
            pso = ps.tile([P, 512], F32, tag="u", name=f"o_ps_{m}_{n}_{qh}",
                          padded_shape=[P, 512])
            qs = slice(n * 512 + qh * nq, n * 512 + qh * nq + nq)
            for c in range(HG):
                nc.tensor.matmul(
                    pso[:, 0:nq],
                    wo_sb[:, c, _ts(m, P)],
                    on_sb[:, c, qs],
                    start=(c == 0), stop=(c == HG - 1),
                )
            st = ow.tile([P, 512], F16, tag="o_st", padded_shape=[P, 512],
                         name=f"o_st_{m}_{n}_{qh}")
            if cast_on_act:
                nc.scalar.copy(st[:, 0:nq], pso[:, 0:nq])
            else:
                nc.vector.tensor_copy(st[:, 0:nq], pso[:, 0:nq])
            eng.dma_start(
                out=outT[:, :][_ts(m, P), qs], in_=st[:, 0:nq],
            )

        # Upfront: only what the first k-tiles of block (0,0) need.  Emitted
        # as interleaved halves so the K half on x(c=0,1) starts while
        # x(c=2,3) is still in flight.
        proj_half(wk_sb, kt_sb, bk_sb, 0, 0, 0)
        proj_half(wq_sb, qt_sb, bq_sb, 0, 0, 0)
        proj_half(wk_sb, kt_sb, bk_sb, 0, 0, 1, on_act=True)
        proj_half(wq_sb, qt_sb, bq_sb, 0, 0, 1)

        def inject(n, pr, i):
            """Emit filler work (projections, previous chunk's output
            projection) AFTER tile i's attention ops so the FIFO PE queue
            serves scores/PV first and the filler soaks up ACT-bound slack."""
            if n == 0 and pr == 0:
                if i == 0:
                    v_proj_half(1, 0)
                    v_proj_half(1, 1)
                elif i == 1:
                    v_proj(2)
                    v_proj(3)
                elif i < 14:
                    v_proj(i + 2)        # V arrives 2 k-tiles ahead
                if i in (0, 1):          # K(t0,n1) needed at k-tile 4
                    proj_half(wk_sb, kt_sb, bk_sb, 0, 1, i)
                if i in (3, 4):
                    proj_half(wk_sb, kt_sb, bk_sb, 0, 2, i - 3)
                if i in (6, 7):
                    proj_half(wk_sb, kt_sb, bk_sb, 0, 3, i - 6)
                if i in (9, 10):
                    proj_half(wk_sb, kt_sb, bk_sb, 1, 0, i - 9)
                if i in (11, 12):
                    proj_half(wq_sb, qt_sb, bq_sb, 1, 0, i - 11)
            elif n == 0 and pr == 1:
                if i in (0, 1):
                    proj_half(wk_sb, kt_sb, bk_sb, 1, 1, i)
                if i in (2, 3):
                    proj_half(wk_sb, kt_sb, bk_sb, 1, 2, i - 2)
                if i in (5, 6):
                    proj_half(wk_sb, kt_sb, bk_sb, 1, 3, i - 5)
                if i in (8, 9):
                    proj_half(wq_sb, qt_sb, bq_sb, 0, 1, i - 8)
            else:
                if i in (5, 6):
                    if pr == 1 and n + 1 < QC:
                        proj_half(wq_sb, qt_sb, bq_sb, 0, n + 1, i - 5)
                    elif pr == 0:
                        proj_half(wq_sb, qt_sb, bq_sb, 1, n, i - 5)
                if pr == 0 and i in (8, 10, 12, 14):
                    out_proj_m(n - 1, (i - 8) // 2)

        def norm_evac(n, pr, o_t, j, nq=512, qh=0):
            # fp32 evacuation: frees the PSUM o bank as fast as possible so
            # the next block's first PV matmul (stuck behind it in the PE
            # FIFO) is not stalled by the rest of the normalize chain
            ocp = nw.tile([DH + 1, 512], F32, tag="ocp", bufs=3,
                          padded_shape=[DH + 1, 512],
                          name=f"ocp_{n}_{pr}_{j}_{qh}")
            nc.vector.tensor_copy(ocp[:, 0:nq],
                                  o_t[j][:, qh * nq:qh * nq + nq])
            return ocp

        def norm_recip(n, pr, ocp, j, nq=512, qh=0):
            # denominator row 64 -> partition 0 (COPY-class op: the DVE
            # reshape front-end can cross partitions; custom DVE ops like
            # reciprocal cannot).  ocp may be the SBUF evacuation (qh=0,
            # full width) or the PSUM accumulator itself (tail, qh-sliced).
            off = 0 if ocp.shape[-1] == 512 and nq == 512 else qh * nq
            dsb = nw.tile([1, 512], F32, tag="dsb", padded_shape=[1, 512],
                          name=f"dsb_{n}_{pr}_{j}_{qh}")
            nc.vector.tensor_copy(dsb[:, 0:nq], ocp[DH:DH + 1, off:off + nq])
            r = nw.tile([1, 512], F32, tag="r", padded_shape=[1, 512],
                        name=f"r_{n}_{pr}_{j}_{qh}")
            nc.vector.reciprocal_approx_fast(r[:, 0:nq], dsb[:, 0:nq])
            # broadcast r across 64 partitions on GPSIMD
            rb = nw.tile([DH, 512], F32, tag="rb", padded_shape=[DH, 512],
                         name=f"rb_{n}_{pr}_{j}_{qh}")
            nc.gpsimd.partition_broadcast(rb[:, 0:nq], r[:, 0:nq])
            return rb

        def norm_mul(n, pr, ocp, rb, j, nq=512, qh=0):
            qs = slice(n * 512 + qh * nq, n * 512 + qh * nq + nq)
            off = 0 if ocp.shape[-1] == 512 and nq == 512 else qh * nq
            nc.vector.tensor_mul(out=on_sb[_ts(j, DH), pr, qs],
                                 in0=ocp[0:DH, off:off + nq],
                                 in1=rb[:, 0:nq])

        def norm_finish(n, pr, ocp, j, nq=512, qh=0):
            rb = norm_recip(n, pr, ocp, j, nq, qh)
            norm_mul(n, pr, ocp, rb, j, nq, qh)

        # ---- attention (64-row PE mode) + interleaved output projection ---
        # The PV matmuls run one tile LATE: each tile's PV is emitted after
        # the NEXT tile's scores+exp, so the ACT queue is never starved while
        # the PE drains PV pairs (matters most at block boundaries).
        pending = None           # (n, pr, i, o_t, et)

        def flush_pending():
            if pending is None:
                return
            pn, ppr, pi, po_t, pet = pending
            for j in range(2):
                nc.tensor.matmul(
                    po_t[j][:],
                    va_sb[:, pi, 2 * ppr + j, :],
                    pet[:, j, :],
                    start=(pi == 0), stop=(pi == KT - 1),
                )
            if pi == KT - 1:
                last = (pn == QC - 1) and (ppr == HG - 1)
                if not last:
                    # both evacuations first (fast PSUM release), then the
                    # lazy normalize chains
                    ocps = [norm_evac(pn, ppr, po_t, j) for j in range(2)]
                    for j in range(2):
                        norm_finish(pn, ppr, ocps[j], j)
                else:
                    # tail: no evacuation (the o banks never recycle at the
                    # end) — normalize straight out of PSUM, stage-ordered,
                    # out-DMAs spread across queues, half the fp16 casts on
                    # the idle ACT engine
                    rbs = {}
                    for qh in range(2):
                        for j in range(2):
                            rbs[qh, j] = norm_recip(pn, ppr, po_t[j], j,
                                                    nq=256, qh=qh)
                    for qh in range(2):
                        for j in range(2):
                            norm_mul(pn, ppr, po_t[j], rbs[qh, j], j,
                                     nq=256, qh=qh)
                        for m in range(4):
                            out_proj_m(pn, m, nq=256, qh=qh,
                                       eng=(nc.sync, nc.scalar)[m % 2],
                                       cast_on_act=(m % 2 == 1))

        for n in range(QC):          # q chunk of 512
            for pr in range(HG):     # head pair (heads 2pr, 2pr+1)
                o_t = [ps.tile([DH + 1, 512], F32, tag="o", bufs=2,
                               name=f"o_{pr}_{n}_{j}")
                       for j in range(2)]
                for i in range(KT):
                    s = ps.tile([P, 2, 512], F32, tag="s", bufs=2,
                                name=f"s_{pr}_{n}_{i}")
                    for j in range(2):
                        nc.tensor.matmul(
                            s[:, j, :],
                            kt_sb[_ts(j, DH), pr, _ts(i, P)],
                            qt_sb[_ts(j, DH), pr, _ts(n, 512)],
                            start=True, stop=True,
                        )
                    et = ew.tile([P, 2, 512], BF16, tag="et", bufs=8)
                    nc.scalar.activation(et[:], s[:], Exp, scale=SCALE)
                    if n == 0 and pr == 0 and i == 0:
                        # va[0] is needed by the first PV; everything later
                        # comes via inject() after each tile
                        v_proj_half(0, 0)
                        v_proj_half(0, 1)
                    flush_pending()
                    pending = (n, pr, i, o_t, et)
                    inject(n, pr, i)
        flush_pending()              # last tile's PV + the tail


_CACHE = {}


def _get_nc():
    if "nc" not in _CACHE:
        # Bacc (not raw Bass): its compile() pipeline legalizes semaphore
        # waits (TRN2 allows at most one wait per instruction).
        nc = bacc.Bacc(None, target_bir_lowering=False)
        with tile.TileContext(nc) as tc:
            _body(tc)
        nc.finalize()
        _CACHE["nc"] = nc
    return _CACHE["nc"]


def make_in_maps(x, Wq, bq, Wk, bk, Wv, bv, Wo):
    bf = ml_dtypes.bfloat16
    in_maps = []
    for c in range(8):
        b, g = c // 2, c % 2
        es = slice(g * EG, (g + 1) * EG)
        bpk = np.zeros((P, 4), np.float32)
        bpk[:, 0] = np.asarray(bq)[es][0:P]
        bpk[:, 1] = np.asarray(bq)[es][P:2 * P]
        bpk[:, 2] = np.asarray(bk)[es][0:P]
        bpk[:, 3] = np.asarray(bk)[es][P:2 * P]
        in_maps.append({
            "xT": np.ascontiguousarray(np.asarray(x)[b].T).astype(bf),
            "wqT": np.ascontiguousarray(np.asarray(Wq)[es, :].T).astype(bf),
            "wkT": np.ascontiguousarray(np.asarray(Wk)[es, :].T).astype(bf),
            "wvT": np.ascontiguousarray(np.asarray(Wv)[es, :].T).astype(bf),
            "woT": np.ascontiguousarray(np.asarray(Wo)[:, es].T).astype(bf),
            "bpk": bpk,
        })
    return in_maps


def gather_out(results, bo, bv, Wo):
    # device partials exclude the V bias: (O/d) @ Wo.  bv passes through the
    # attention untouched (softmax rows sum to 1), so its contribution is the
    # constant vector bv @ Wo.T, added here with bo.
    const = (np.asarray(bo, np.float64)
             + np.asarray(bv, np.float64) @ np.asarray(Wo, np.float64).T
             ).astype(np.float32)
    out = np.empty((B, L, D), np.float32)
    for b in range(B):
        out[b] = (results[2 * b]["outT"].astype(np.float32).T
                  + results[2 * b + 1]["outT"].astype(np.float32).T
                  + const[None, :])
    return out


def kernel(x, Wq, bq, Wk, bk, Wv, bv, Wo, bo, **kwargs):
    nc = _get_nc()
    in_maps = make_in_maps(x, Wq, bq, Wk, bk, Wv, bv, Wo)
    res = run_bass_kernel_spmd(nc, in_maps, list(range(8)))
    return gather_out(res.results, bo, bv, Wo)
